# revision 1
# baseline (speedup 1.0000x reference)
"""Multi-head causal attention (B=4, S=2048, D=1024, H=16) on 8 trn2 cores.

Sharding: tensor-parallel over heads x data-parallel over batch.
core c -> (batch b = c//2, head-group hg = c%2 of 8 heads). Every core runs
an identical SPMD program on different data:
  - QKV projections for its 512 features (8 heads). K kept transposed
    [feat, seq] in SBUF, V kept [seq, feat] with an appended ones column per
    head (softmax denominators come free out of the PV matmul), Q produced
    per 512-query superblock just in time.
  - Causal attention per (head, superblock): S^T = K^T.T @ Q^T per 128-key
    block as two N=256 matmuls (f32r K=64/M=128 is half-rate at N=512), exp
    on ScalarE with no max subtraction (scores are O(5), exp cannot
    overflow), 0/1 mask multiply on diagonal blocks, PV accumulation in
    PSUM with an M=128-padded stationary.
  - Output projection against the head-group's 512-column slice of Wo.
Host sums the two partial outputs per batch (the "all-reduce after W_o"
done at gather time) and folds the Wo @ bv + bo constant.

Two trn2-specific tricks:
  - All matmuls run in float32r (11-bit mantissa, 4x fp32 PE rate); inputs
    are pre-rounded on the host (RNE at bit 12) so DMA feeds matmul tiles
    directly.
  - The PE HAM clock-gate does not count K=64 matmuls as "busy", so a pure
    attention phase runs at 1.2 GHz. The projection and output-projection
    chains (K=128) are therefore interleaved INTO the attention stream,
    which keeps the clock at 2.4 GHz: projections for superblock sc+1 and
    the output projection for sc-1 are emitted between attention batches
    of superblock sc.
"""

import sys

import numpy as np

_BASS_PATH = "/opt/trn_rl_repo"
if _BASS_PATH not in sys.path:
    sys.path.insert(0, _BASS_PATH)

B, S, D, H, DK = 4, 2048, 1024, 16, 64
NCORES = 8
FH = 512  # features per core (8 heads)
HL = 8  # local heads
NSC = 4  # seq superblocks of 512
SQ = 512
NKB = 16  # key blocks of 128
NDM = 8  # d_model chunks of 128

_cache = {}


def _round_f32r(x: np.ndarray) -> np.ndarray:
    """Round fp32 to fp32r (RNE to 11 mantissa bits) - matches TRN2 HW."""
    v = np.ascontiguousarray(x, dtype=np.float32).view(np.uint32)
    lsb = (v >> np.uint32(12)) & np.uint32(1)
    out = ((v + np.uint32(0x7FF) + lsb) >> np.uint32(12)) << np.uint32(12)
    return out.view(np.float32)


def _build():
    import concourse.bacc as bacc
    import concourse.mybir as mybir
    from concourse.tile import TileContext

    f32, f32r = mybir.dt.float32, mybir.dt.float32r
    AF = mybir.ActivationFunctionType

    nc = bacc.Bacc("TRN2", target_bir_lowering=False, debug=False, num_devices=1)

    xq_d = nc.dram_tensor("xq", [D, S], f32r, kind="ExternalInput").ap()
    xk_d = nc.dram_tensor("xk", [D, S], f32r, kind="ExternalInput").ap()
    xv_d = nc.dram_tensor("xv", [D, S], f32r, kind="ExternalInput").ap()
    wq_d = nc.dram_tensor("wq", [D, FH], f32r, kind="ExternalInput").ap()
    wk_d = nc.dram_tensor("wk", [D, FH], f32r, kind="ExternalInput").ap()
    wv_d = nc.dram_tensor("wv", [D, FH], f32r, kind="ExternalInput").ap()
    wo_d = nc.dram_tensor("wo", [FH, D], f32r, kind="ExternalInput").ap()
    # master causal mask [128, 896]: m[k, c] = 1 iff k <= c - 384.
    # mask_j (j = diag block index) = master[:, (3-j)*128 : (3-j)*128+512]
    mask_d = nc.dram_tensor("masks", [128, 896], f32r, kind="ExternalInput").ap()
    bq_d = nc.dram_tensor("bq", [FH], f32, kind="ExternalInput").ap()
    bk_d = nc.dram_tensor("bk", [FH], f32, kind="ExternalInput").ap()
    out_d = nc.dram_tensor("out", [S, D], f32, kind="ExternalOutput").ap()

    with TileContext(nc) as tc:
        with (
            tc.tile_pool(name="res", bufs=1) as res,
            tc.tile_pool(name="st", bufs=1) as st,
            tc.tile_pool(name="psum", bufs=1, space="PSUM") as psp,
            tc.tile_pool(name="dram", bufs=1, space="DRAM") as dpool,
        ):
            kt = [res.tile([128, S], f32r, name=f"kt{i}", tag=f"kt{i}") for i in range(4)]
            # 520 data cols (8 heads x (64 V + ones)) + pad so PV can read a
            # 128-wide stationary slice for head 7 (rows 65+ of the PV output
            # are garbage and ignored)
            vaug = [
                res.tile([128, 584], f32r, name=f"va{k}", tag=f"va{k}")
                for k in range(NKB)
            ]
            master = res.tile([128, 896], f32r, name="master", tag="master")
            nc.sync.dma_start(master[:], mask_d[:])
            bq_t = [res.tile([128, 1], f32, name=f"bq{i}", tag=f"bq{i}") for i in range(4)]
            bk_t = [res.tile([128, 1], f32, name=f"bk{i}", tag=f"bk{i}") for i in range(4)]
            for i in range(4):
                nc.sync.dma_start(
                    bq_t[i][:],
                    bq_d[i * 128 : (i + 1) * 128].rearrange("(p o) -> p o", o=1),
                )
                nc.sync.dma_start(
                    bk_t[i][:],
                    bk_d[i * 128 : (i + 1) * 128].rearrange("(p o) -> p o", o=1),
                )
            ones_t = res.tile([128, HL], f32, name="ones", tag="ones")
            nc.vector.memset(ones_t[:], 1.0)
            wo_sb = []
            for fc in range(4):
                wt = res.tile([128, D], f32r, name=f"wo{fc}", tag=f"wo{fc}")
                nc.sync.dma_start(wt[:], wo_d[fc * 128 : (fc + 1) * 128, :])
                wo_sb.append(wt)
            ctxd = dpool.tile([FH, S], f32r, name="ctxd", tag="ctxd")

            qsf_box = {}  # sc -> [4 q-slice tiles]

            def make_proj_thunks(sc):
                thunks = []
                for pname, x_d, w_d in (
                    ("k", xk_d, wk_d),
                    ("v", xv_d, wv_d),
                    ("q", xq_d, wq_d),
                ):
                    box = {}

                    def load(pname=pname, x_d=x_d, w_d=w_d, box=box):
                        w_sb, xr = [], []
                        for dm in range(NDM):
                            wt = st.tile(
                                [128, FH], f32r, name=f"w{dm}", tag=f"w{dm}", bufs=2
                            )
                            nc.sync.dma_start(wt[:], w_d[dm * 128 : (dm + 1) * 128, :])
                            w_sb.append(wt)
                            xt = st.tile(
                                [128, SQ], f32r, name=f"x{dm}", tag=f"x{dm}", bufs=1
                            )
                            nc.sync.dma_start(
                                xt[:],
                                x_d[dm * 128 : (dm + 1) * 128, sc * SQ : (sc + 1) * SQ],
                            )
                            xr.append(xt)
                        box["w"], box["x"] = w_sb, xr
                        if pname == "q":
                            qsf_box[sc] = [
                                st.tile(
                                    [128, SQ], f32r, name=f"qs{i}", tag=f"qs{i}", bufs=2
                                )
                                for i in range(4)
                            ]

                    for gi in range(4):

                        def group(pname=pname, gi=gi, box=box, sc=sc, load=load):
                            if gi == 0:
                                load()
                            w_sb, xr = box["w"], box["x"]
                            if pname in ("q", "k"):
                                pp = psp.tile(
                                    [128, SQ], f32, name="pp", tag="pp", bufs=2
                                )
                                for dm in range(NDM):
                                    nc.tensor.matmul(
                                        pp[:],
                                        w_sb[dm][:, gi * 128 : (gi + 1) * 128],
                                        xr[dm][:],
                                        start=(dm == 0),
                                        stop=(dm == NDM - 1),
                                    )
                                if pname == "k":
                                    nc.scalar.activation(
                                        kt[gi][:, sc * SQ : (sc + 1) * SQ],
                                        pp[:],
                                        AF.Identity,
                                        bias=bk_t[gi][:],
                                    )
                                else:
                                    nc.scalar.activation(
                                        qsf_box[sc][gi][:],
                                        pp[:],
                                        AF.Identity,
                                        bias=bq_t[gi][:],
                                    )
                            else:  # v
                                kb = sc * 4 + gi
                                pp = psp.tile(
                                    [128, FH], f32, name="pp", tag="pp", bufs=2
                                )
                                for dm in range(NDM):
                                    nc.tensor.matmul(
                                        pp[:],
                                        xr[dm][:, gi * 128 : (gi + 1) * 128],
                                        w_sb[dm][:],
                                        start=(dm == 0),
                                        stop=(dm == NDM - 1),
                                    )
                                va3 = vaug[kb][:, 0 : HL * 65].rearrange(
                                    "p (h e) -> p h e", e=65
                                )
                                pp3 = pp[:].rearrange("p (h e) -> p h e", e=64)
                                nc.vector.tensor_copy(va3[:, :, 0:64], pp3[:])
                                nc.vector.tensor_copy(
                                    va3[:, :, 64:65],
                                    ones_t[:].rearrange("p (h o) -> p h o", o=1),
                                )

                        thunks.append(group)
                return thunks

            def make_attn_batches(h, sb):
                """Return list of batch thunks for one (head, superblock)."""
                ti, po = h // 2, (h % 2) * 64
                nkb = 4 * (sb + 1)
                kbs = list(range(4 * sb, 4 * sb + 4)) + list(range(4 * sb))
                state = {}

                def batch(b0):
                    if b0 == 0:
                        state["cp"] = psp.tile(
                            [128, SQ], f32, name="cp", tag="cp", bufs=2
                        )
                        state["emitted"] = 0
                    cp = state["cp"]
                    group = []
                    for i in range(b0, b0 + 4):
                        kb = kbs[i]
                        sp = psp.tile([128, SQ], f32, name="sp", tag="sp", bufs=4)
                        for n0 in (0, 256):
                            nc.tensor.matmul(
                                sp[:, n0 : n0 + 256],
                                kt[ti][po : po + 64, kb * 128 : (kb + 1) * 128],
                                qsf_box[sb][ti][po : po + 64, n0 : n0 + 256],
                                start=True,
                                stop=True,
                            )
                        es = st.tile([128, SQ], f32r, name="es", tag="es", bufs=5)
                        nc.scalar.activation(es[:], sp[:], AF.Exp)
                        if kb >= sb * 4:
                            j = kb - sb * 4
                            es2 = st.tile(
                                [128, SQ], f32r, name="es2", tag="es2", bufs=5
                            )
                            nc.vector.tensor_mul(
                                es2[:],
                                es[:],
                                master[:, (3 - j) * 128 : (3 - j) * 128 + 512],
                            )
                            es = es2
                        group.append((kb, es))
                    for off in reversed(range(4)):
                        kb, es = group[off]
                        nc.tensor.matmul(
                            cp[:],
                            vaug[kb][:, h * 65 : h * 65 + 128],
                            es[:],
                            start=(state["emitted"] == 0),
                            stop=(state["emitted"] == nkb - 1),
                        )
                        state["emitted"] += 1
                    if b0 + 4 >= nkb:
                        # normalize and spill ctx^T slice to DRAM
                        d1 = st.tile([1, SQ], f32, name="d1", tag="d1", bufs=2)
                        nc.scalar.copy(d1[:], cp[64:65, :])
                        rb = st.tile([64, SQ], f32, name="rb", tag="rb", bufs=2)
                        nc.gpsimd.partition_broadcast(rb[:], d1[:])
                        rc = st.tile([64, SQ], f32, name="rc", tag="rc", bufs=2)
                        nc.vector.reciprocal_approx_fast(rc[:], rb[:])
                        nrm = st.tile([64, SQ], f32r, name="nrm", tag="nrm", bufs=2)
                        nc.vector.tensor_mul(nrm[:], cp[0:64, :], rc[:])
                        nc.sync.dma_start(
                            ctxd[h * 64 : (h + 1) * 64, sb * SQ : (sb + 1) * SQ],
                            nrm[:],
                        )

                return [
                    (lambda b0=b0: batch(b0)) for b0 in range(0, nkb, 4)
                ]

            def make_o_thunks(sb):
                thunks = []
                box = {}

                def load(sb=sb, box=box):
                    cfc = []
                    for fc in range(4):
                        ct = st.tile(
                            [128, SQ], f32r, name=f"cf{fc}", tag=f"cf{fc}", bufs=1
                        )
                        nc.sync.dma_start(
                            ct[:],
                            ctxd[fc * 128 : (fc + 1) * 128, sb * SQ : (sb + 1) * SQ],
                        )
                        cfc.append(ct)
                    box["c"] = cfc

                for qb in range(4):
                    for n2 in range(2):

                        def group(qb=qb, n2=n2, sb=sb, box=box):
                            if qb == 0 and n2 == 0:
                                load()
                            cfc = box["c"]
                            pp = psp.tile([128, SQ], f32, name="pp", tag="pp", bufs=2)
                            for fc in range(4):
                                nc.tensor.matmul(
                                    pp[:],
                                    cfc[fc][:, qb * 128 : (qb + 1) * 128],
                                    wo_sb[fc][:, n2 * SQ : (n2 + 1) * SQ],
                                    start=(fc == 0),
                                    stop=(fc == 3),
                                )
                            ob = st.tile([128, SQ], f32, name="ob", tag="ob", bufs=2)
                            nc.vector.tensor_copy(ob[:], pp[:])
                            nc.sync.dma_start(
                                out_d[
                                    sb * SQ + qb * 128 : sb * SQ + (qb + 1) * 128,
                                    n2 * SQ : (n2 + 1) * SQ,
                                ],
                                ob[:],
                            )

                        thunks.append(group)
                return thunks

            dummy_state = {"n": 0}

            def make_dummy_thunks(n):
                thunks = []
                for _ in range(n):

                    def g():
                        dp = psp.tile([128, SQ], f32, name="dp", tag="pp", bufs=2)
                        for t in range(4):
                            nc.tensor.matmul(
                                dp[:],
                                wo_sb[t][:, 0:128],
                                wo_sb[(t + 1) % 4][:, 0:SQ],
                                start=(t == 0),
                                stop=(t == 3),
                            )

                    thunks.append(g)
                return thunks

            # ---- emission schedule ----
            for t in make_proj_thunks(0):
                t()
            for sb in range(NSC):
                batches = []
                for h in range(HL):
                    batches += make_attn_batches(h, sb)
                warm = []
                if sb < NSC - 1:
                    warm += make_proj_thunks(sb + 1)
                if sb >= 1:
                    warm += make_o_thunks(sb - 1)
                # pad the warm stream so ~1 in 4 PE chains is K=128 (keeps
                # the HAM clock-gate at full rate through the attention tail)
                want = (len(batches) - len(warm)) // 3
                if want > 0:
                    warm += make_dummy_thunks(want)
                    # re-spread: alternate real and dummy warm items
                    real = warm[: len(warm) - want]
                    dum = warm[len(warm) - want :]
                    mixed = []
                    di = 0
                    for i, w in enumerate(real):
                        mixed.append(w)
                        while di < len(dum) and (di + 1) * len(real) <= (i + 1) * len(dum):
                            mixed.append(dum[di])
                            di += 1
                    mixed += dum[di:]
                    warm = mixed
                nb, nw = len(batches), len(warm)
                wi = 0
                for bi, bt in enumerate(batches):
                    bt()
                    while wi < nw and (wi + 1) * nb <= (bi + 1) * nw:
                        warm[wi]()
                        wi += 1
                while wi < nw:
                    warm[wi]()
                    wi += 1
            for t in make_o_thunks(NSC - 1):
                t()

    nc.compile()
    return nc


def kernel(
    q,
    k,
    v,
    mask=None,
    Wq=None,
    bq=None,
    Wk=None,
    bk=None,
    Wv=None,
    bv=None,
    Wo=None,
    bo=None,
    **_unused,
):
    from concourse.bass_utils import run_bass_kernel_spmd

    if "nc" not in _cache:
        _cache["nc"] = _build()
    nc = _cache["nc"]

    q = np.asarray(q, np.float32)
    k = np.asarray(k, np.float32)
    v = np.asarray(v, np.float32)
    Wq = np.asarray(Wq, np.float32)
    Wk = np.asarray(Wk, np.float32)
    Wv = np.asarray(Wv, np.float32)
    Wo = np.asarray(Wo, np.float32)
    bq = np.zeros(D, np.float32) if bq is None else np.asarray(bq, np.float32)
    bk = np.zeros(D, np.float32) if bk is None else np.asarray(bk, np.float32)
    bv = np.zeros(D, np.float32) if bv is None else np.asarray(bv, np.float32)
    bo = np.zeros(D, np.float32) if bo is None else np.asarray(bo, np.float32)

    qr, kr, vr = _round_f32r(q), _round_f32r(k), _round_f32r(v)
    Wqr, Wkr, Wvr, Wor = map(_round_f32r, (Wq, Wk, Wv, Wo))

    # master causal mask: m[kk, c] = 1 iff kk <= c - 384
    kk = np.arange(128)[:, None]
    cc = np.arange(896)[None, :]
    masks = (kk <= cc - 384).astype(np.float32)

    xT = {}
    for b in range(B):
        xT[("q", b)] = np.ascontiguousarray(qr[b].T)
        xT[("k", b)] = np.ascontiguousarray(kr[b].T)
        xT[("v", b)] = np.ascontiguousarray(vr[b].T)
    wqs, wks, wvs, wos, bqs, bks = {}, {}, {}, {}, {}, {}
    for hg in range(2):
        sl = slice(hg * FH, (hg + 1) * FH)
        wqs[hg] = np.ascontiguousarray(Wqr[sl, :].T) * np.float32(0.125)
        wks[hg] = np.ascontiguousarray(Wkr[sl, :].T)
        wvs[hg] = np.ascontiguousarray(Wvr[sl, :].T)
        wos[hg] = np.ascontiguousarray(Wor[:, sl].T)
        bqs[hg] = np.ascontiguousarray(bq[sl]) * np.float32(0.125)
        bks[hg] = np.ascontiguousarray(bk[sl])

    in_maps = []
    for c in range(NCORES):
        b, hg = c // 2, c % 2
        in_maps.append(
            {
                "xq": xT[("q", b)],
                "xk": xT[("k", b)],
                "xv": xT[("v", b)],
                "wq": wqs[hg],
                "wk": wks[hg],
                "wv": wvs[hg],
                "wo": wos[hg],
                "masks": masks,
                "bq": bqs[hg],
                "bk": bks[hg],
            }
        )

    res = run_bass_kernel_spmd(nc, in_maps, list(range(NCORES)))
    out = np.empty((B, S, D), np.float32)
    for b in range(B):
        out[b] = res.results[2 * b]["out"] + res.results[2 * b + 1]["out"]
    const = Wo @ bv + bo  # bv/bo contribution (folds exactly through softmax)
    if np.any(const):
        out += const[None, None, :]
    return out



# revision 3
# speedup vs baseline: 1.3567x; 1.3567x over previous
"""Multi-head causal attention (B=4, S=2048, D=1024, H=16) on 8 trn2 cores.

Sharding: tensor-parallel over heads x data-parallel over batch.
core c -> (batch b = c//2, head-group hg = c%2 of 8 heads). Every core runs
an identical SPMD program on different data; the host sums the two partial
outputs per batch (the "all-reduce after W_o" done at gather time) and folds
the Wo @ bv + bo constant.

v2 design (vs the 480us f32r baseline):
  - All matmul inputs are bf16 (tolerance 2e-2 leaves plenty of margin);
    halves DMA traffic and doubles DVE throughput on 16-bit tiles.
  - Every matmul is K=128 so the PE HAM clock-gate never throttles (the
    baseline lost 127us to K=4/8 oscillation because K=64 score matmuls
    don't count as "busy"). Score matmuls get K=128 via per-head Q tiles
    zero-padded in the other head's 64 feature rows.
  - QKV/O weights + K^T/V~/Q/ctx all SBUF-resident: weights are loaded
    once (baseline re-loaded 6MB of weights every superblock = +18MB DMA)
    and ctx never round-trips through DRAM.
  - Causal trimming: for diagonal key-block j, scores/exp/PV only cover
    query columns >= 128*j. The fine triangular mask is applied by
    accumulating identity.T @ maskbias (0/-30) into the score PSUM before
    exp - no vector-engine mask multiply at all.
  - Softmax denominators still come free as PV row 64 via a ones column
    appended to each head's V block (stationary M=65).
"""

import sys

import numpy as np

_BASS_PATH = "/opt/trn_rl_repo"
if _BASS_PATH not in sys.path:
    sys.path.insert(0, _BASS_PATH)

B, S, D, H, DK = 4, 2048, 1024, 16, 64
NCORES = 8
FH = 512  # features per core (8 heads)
HL = 8  # local heads
NSC = 4  # seq superblocks of 512
SQ = 512
NDM = 8  # d_model chunks of 128
NEGB = -30.0  # causal mask bias (exp(-30+s) ~ 0 for |s|<=8)

_cache = {}


def _build():
    import concourse.bacc as bacc
    import concourse.mybir as mybir
    from concourse.tile import TileContext

    f32, bf16 = mybir.dt.float32, mybir.dt.bfloat16
    AF = mybir.ActivationFunctionType

    nc = bacc.Bacc("TRN2", target_bir_lowering=False, debug=False, num_devices=1)

    xq_d = nc.dram_tensor("xq", [D, S], bf16, kind="ExternalInput").ap()
    xk_d = nc.dram_tensor("xk", [D, S], bf16, kind="ExternalInput").ap()
    xv_d = nc.dram_tensor("xv", [D, S], bf16, kind="ExternalInput").ap()
    wq_d = nc.dram_tensor("wq", [D, FH], bf16, kind="ExternalInput").ap()
    wk_d = nc.dram_tensor("wk", [D, FH], bf16, kind="ExternalInput").ap()
    wv_d = nc.dram_tensor("wv", [D, FH], bf16, kind="ExternalInput").ap()
    wo_d = nc.dram_tensor("wo", [FH, D], bf16, kind="ExternalInput").ap()
    id_d = nc.dram_tensor("ident", [128, 128], bf16, kind="ExternalInput").ap()
    mb_d = nc.dram_tensor("maskb", [128, 128], bf16, kind="ExternalInput").ap()
    bq_d = nc.dram_tensor("bq", [FH], f32, kind="ExternalInput").ap()
    bk_d = nc.dram_tensor("bk", [FH], f32, kind="ExternalInput").ap()
    out_d = nc.dram_tensor("out", [S, D], f32, kind="ExternalOutput").ap()

    with TileContext(nc) as tc:
        with (
            tc.tile_pool(name="res", bufs=1) as res,
            tc.tile_pool(name="st", bufs=1) as st,
            tc.tile_pool(name="psum", bufs=1, space="PSUM") as psp,
        ):
            # ---- resident tiles ----
            kt = [res.tile([128, S], bf16, name=f"kt{i}", tag=f"kt{i}") for i in range(4)]
            # V~ per key block: 8 heads x (64 V cols + ones col), stride 65
            vaug = [
                res.tile([128, 520], bf16, name=f"va{k}", tag=f"va{k}")
                for k in range(16)
            ]
            wk_sb = [res.tile([128, FH], bf16, name=f"wk{i}", tag=f"wk{i}") for i in range(NDM)]
            wv_sb = [res.tile([128, FH], bf16, name=f"wv{i}", tag=f"wv{i}") for i in range(NDM)]
            wq_sb = [res.tile([128, FH], bf16, name=f"wq{i}", tag=f"wq{i}") for i in range(NDM)]
            w_res = {"k": wk_sb, "v": wv_sb, "q": wq_sb}
            wo_sb = []
            for fc in range(4):
                wt = res.tile([128, D], bf16, name=f"wo{fc}", tag=f"wo{fc}")
                nc.sync.dma_start(wt[:], wo_d[fc * 128 : (fc + 1) * 128, :])
                wo_sb.append(wt)
            for dm in range(NDM):
                nc.sync.dma_start(wk_sb[dm][:], wk_d[dm * 128 : (dm + 1) * 128, :])
                nc.sync.dma_start(wv_sb[dm][:], wv_d[dm * 128 : (dm + 1) * 128, :])
                nc.sync.dma_start(wq_sb[dm][:], wq_d[dm * 128 : (dm + 1) * 128, :])
            ident_t = res.tile([128, 128], bf16, name="ident", tag="ident")
            maskb_t = res.tile([128, 128], bf16, name="maskb", tag="maskb")
            nc.sync.dma_start(ident_t[:], id_d[:])
            nc.sync.dma_start(maskb_t[:], mb_d[:])
            bq_t = [res.tile([128, 1], f32, name=f"bq{i}", tag=f"bq{i}") for i in range(4)]
            bk_t = [res.tile([128, 1], f32, name=f"bk{i}", tag=f"bk{i}") for i in range(4)]
            for i in range(4):
                nc.sync.dma_start(
                    bq_t[i][:],
                    bq_d[i * 128 : (i + 1) * 128].rearrange("(p o) -> p o", o=1),
                )
                nc.sync.dma_start(
                    bk_t[i][:],
                    bk_d[i * 128 : (i + 1) * 128].rearrange("(p o) -> p o", o=1),
                )
            ones_t = res.tile([128, HL], bf16, name="ones", tag="ones")
            nc.vector.memset(ones_t[:], 1.0)
            # per-head zero-padded Q tiles, double-buffered over superblocks:
            # head h data lives in rows (h%2)*64 .. +64, other 64 rows are 0
            qz = [
                [
                    res.tile([128, SQ], bf16, name=f"qz{s}_{h}", tag=f"qz{s}_{h}")
                    for h in range(HL)
                ]
                for s in range(2)
            ]
            for s in range(2):
                for h in range(HL):
                    zr = 64 if (h % 2 == 0) else 0
                    nc.vector.memset(qz[s][h][zr : zr + 64, :], 0.0)
            # SBUF-resident ctx^T (features x queries), double-buffered
            cfs = [
                [
                    res.tile([128, SQ], bf16, name=f"cf{s}_{fc}", tag=f"cf{s}_{fc}")
                    for fc in range(4)
                ]
                for s in range(2)
            ]

            def make_proj_thunks(sc):
                thunks = []
                for pname, x_d in (("k", xk_d), ("v", xv_d), ("q", xq_d)):
                    box = {}

                    def load(pname=pname, x_d=x_d, box=box):
                        xr = []
                        for dm in range(NDM):
                            xt = st.tile(
                                [128, SQ], bf16, name=f"x{dm}", tag=f"x{dm}", bufs=1
                            )
                            nc.sync.dma_start(
                                xt[:],
                                x_d[dm * 128 : (dm + 1) * 128, sc * SQ : (sc + 1) * SQ],
                            )
                            xr.append(xt)
                        box["x"] = xr

                    for gi in range(4):

                        def group(pname=pname, gi=gi, box=box, sc=sc, load=load):
                            if gi == 0:
                                load()
                            w_sb = w_res[pname]
                            xr = box["x"]
                            if pname in ("q", "k"):
                                pp = psp.tile(
                                    [128, SQ], f32, name="pp", tag="pp", bufs=2
                                )
                                for dm in range(NDM):
                                    nc.tensor.matmul(
                                        pp[:],
                                        w_sb[dm][:, gi * 128 : (gi + 1) * 128],
                                        xr[dm][:],
                                        start=(dm == 0),
                                        stop=(dm == NDM - 1),
                                    )
                                if pname == "k":
                                    nc.scalar.activation(
                                        kt[gi][:, sc * SQ : (sc + 1) * SQ],
                                        pp[:],
                                        AF.Identity,
                                        bias=bk_t[gi][:],
                                    )
                                else:
                                    s = sc % 2
                                    nc.scalar.activation(
                                        qz[s][2 * gi][0:64, :],
                                        pp[0:64, :],
                                        AF.Identity,
                                        bias=bq_t[gi][0:64],
                                    )
                                    nc.scalar.activation(
                                        qz[s][2 * gi + 1][64:128, :],
                                        pp[64:128, :],
                                        AF.Identity,
                                        bias=bq_t[gi][64:128],
                                    )
                            else:  # v
                                kb = sc * 4 + gi
                                pp = psp.tile(
                                    [128, FH], f32, name="pp", tag="pp", bufs=2
                                )
                                for dm in range(NDM):
                                    nc.tensor.matmul(
                                        pp[:],
                                        xr[dm][:, gi * 128 : (gi + 1) * 128],
                                        w_sb[dm][:],
                                        start=(dm == 0),
                                        stop=(dm == NDM - 1),
                                    )
                                va3 = vaug[kb][:, 0 : HL * 65].rearrange(
                                    "p (h e) -> p h e", e=65
                                )
                                pp3 = pp[:].rearrange("p (h e) -> p h e", e=64)
                                nc.vector.tensor_copy(va3[:, :, 0:64], pp3[:])
                                nc.vector.tensor_copy(
                                    va3[:, :, 64:65],
                                    ones_t[:].rearrange("p (h o) -> p h o", o=1),
                                )

                        thunks.append(group)
                return thunks

            def make_attn_batches(h, sb):
                """Batches of 4 key-blocks for one (head, superblock)."""
                ti = h // 2
                nkb = 4 * (sb + 1)
                # off-diagonal blocks first (full N), then diagonal ascending
                kbs = list(range(4 * sb)) + list(range(4 * sb, 4 * sb + 4))
                state = {}

                def batch(b0):
                    if b0 == 0:
                        state["cp"] = psp.tile(
                            [128, SQ], f32, name="cp", tag="cp", bufs=2
                        )
                        state["emitted"] = 0
                    cp = state["cp"]
                    group = []
                    for i in range(b0, b0 + 4):
                        kb = kbs[i]
                        j = kb - 4 * sb  # >=0 on diagonal blocks
                        c0 = 128 * j if j >= 0 else 0
                        sp = psp.tile([128, SQ], f32, name="sp", tag="sp", bufs=4)
                        nc.tensor.matmul(
                            sp[:, c0:SQ],
                            kt[ti][:, kb * 128 : (kb + 1) * 128],
                            qz[sb % 2][h][:, c0:SQ],
                            start=True,
                            stop=(j < 0),
                        )
                        if j >= 0:
                            nc.tensor.matmul(
                                sp[:, c0 : c0 + 128],
                                ident_t[:],
                                maskb_t[:],
                                start=False,
                                stop=True,
                            )
                        es = st.tile([128, SQ], bf16, name="es", tag="es", bufs=6)
                        nc.scalar.activation(es[:, c0:SQ], sp[:, c0:SQ], AF.Exp)
                        group.append((kb, c0, es))
                    for kb, c0, es in group:
                        nc.tensor.matmul(
                            cp[0:65, c0:SQ],
                            vaug[kb][:, h * 65 : h * 65 + 65],
                            es[:, c0:SQ],
                            start=(state["emitted"] == 0),
                            stop=(state["emitted"] == nkb - 1),
                        )
                        state["emitted"] += 1
                    if b0 + 4 >= nkb:
                        # normalize by PV row 64 and park ctx^T slice in SBUF
                        d1 = st.tile([1, SQ], f32, name="d1", tag="d1", bufs=2)
                        nc.scalar.copy(d1[:], cp[64:65, :])
                        rc1 = st.tile([1, SQ], f32, name="rc1", tag="rc1", bufs=2)
                        nc.vector.reciprocal_approx_fast(rc1[:], d1[:])
                        rb = st.tile([64, SQ], f32, name="rb", tag="rb", bufs=2)
                        nc.gpsimd.partition_broadcast(rb[:], rc1[:])
                        if h % 2 == 0:
                            nc.vector.tensor_mul(
                                cfs[sb % 2][ti][0:64, :], cp[0:64, :], rb[:]
                            )
                        else:
                            nrm = st.tile(
                                [64, SQ], bf16, name="nrm", tag="nrm", bufs=2
                            )
                            nc.vector.tensor_mul(nrm[:], cp[0:64, :], rb[:])
                            nc.sync.dma_start(cfs[sb % 2][ti][64:128, :], nrm[:])

                return [(lambda b0=b0: batch(b0)) for b0 in range(0, nkb, 4)]

            def make_o_thunks(sb):
                thunks = []
                for qb in range(4):
                    for n2 in range(2):

                        def group(qb=qb, n2=n2, sb=sb):
                            cfc = cfs[sb % 2]
                            pp = psp.tile([128, SQ], f32, name="pp", tag="pp", bufs=2)
                            for fc in range(4):
                                nc.tensor.matmul(
                                    pp[:],
                                    cfc[fc][:, qb * 128 : (qb + 1) * 128],
                                    wo_sb[fc][:, n2 * SQ : (n2 + 1) * SQ],
                                    start=(fc == 0),
                                    stop=(fc == 3),
                                )
                            ob = st.tile([128, SQ], f32, name="ob", tag="ob", bufs=2)
                            nc.vector.tensor_copy(ob[:], pp[:])
                            nc.sync.dma_start(
                                out_d[
                                    sb * SQ + qb * 128 : sb * SQ + (qb + 1) * 128,
                                    n2 * SQ : (n2 + 1) * SQ,
                                ],
                                ob[:],
                            )

                        thunks.append(group)
                return thunks

            # ---- emission schedule ----
            for t in make_proj_thunks(0):
                t()
            for sb in range(NSC):
                batches = []
                for h in range(HL):
                    batches += make_attn_batches(h, sb)
                warm = []
                if sb < NSC - 1:
                    warm += make_proj_thunks(sb + 1)
                if sb >= 1:
                    warm += make_o_thunks(sb - 1)
                nb, nw = len(batches), len(warm)
                wi = 0
                for bi, bt in enumerate(batches):
                    bt()
                    while wi < nw and (wi + 1) * nb <= (bi + 1) * nw:
                        warm[wi]()
                        wi += 1
                while wi < nw:
                    warm[wi]()
                    wi += 1
            for t in make_o_thunks(NSC - 1):
                t()

    nc.compile()
    return nc


def kernel(
    q,
    k,
    v,
    mask=None,
    Wq=None,
    bq=None,
    Wk=None,
    bk=None,
    Wv=None,
    bv=None,
    Wo=None,
    bo=None,
    **_unused,
):
    import ml_dtypes

    from concourse.bass_utils import run_bass_kernel_spmd

    if "nc" not in _cache:
        _cache["nc"] = _build()
    nc = _cache["nc"]

    bf16 = ml_dtypes.bfloat16
    q = np.asarray(q, np.float32)
    k = np.asarray(k, np.float32)
    v = np.asarray(v, np.float32)
    Wq = np.asarray(Wq, np.float32)
    Wk = np.asarray(Wk, np.float32)
    Wv = np.asarray(Wv, np.float32)
    Wo = np.asarray(Wo, np.float32)
    bq = np.zeros(D, np.float32) if bq is None else np.asarray(bq, np.float32)
    bk = np.zeros(D, np.float32) if bk is None else np.asarray(bk, np.float32)
    bv = np.zeros(D, np.float32) if bv is None else np.asarray(bv, np.float32)
    bo = np.zeros(D, np.float32) if bo is None else np.asarray(bo, np.float32)

    ident = np.eye(128, dtype=np.float32).astype(bf16)
    kk = np.arange(128)[:, None]
    qq = np.arange(128)[None, :]
    maskb = np.where(kk <= qq, 0.0, NEGB).astype(bf16)

    xT = {}
    for b in range(B):
        xT[("q", b)] = np.ascontiguousarray(q[b].T).astype(bf16)
        xT[("k", b)] = np.ascontiguousarray(k[b].T).astype(bf16)
        xT[("v", b)] = np.ascontiguousarray(v[b].T).astype(bf16)
    wqs, wks, wvs, wos, bqs, bks = {}, {}, {}, {}, {}, {}
    for hg in range(2):
        sl = slice(hg * FH, (hg + 1) * FH)
        wqs[hg] = (np.ascontiguousarray(Wq[sl, :].T) * np.float32(0.125)).astype(bf16)
        wks[hg] = np.ascontiguousarray(Wk[sl, :].T).astype(bf16)
        wvs[hg] = np.ascontiguousarray(Wv[sl, :].T).astype(bf16)
        wos[hg] = np.ascontiguousarray(Wo[:, sl].T).astype(bf16)
        bqs[hg] = np.ascontiguousarray(bq[sl]) * np.float32(0.125)
        bks[hg] = np.ascontiguousarray(bk[sl])

    in_maps = []
    for c in range(NCORES):
        b, hg = c // 2, c % 2
        in_maps.append(
            {
                "xq": xT[("q", b)],
                "xk": xT[("k", b)],
                "xv": xT[("v", b)],
                "wq": wqs[hg],
                "wk": wks[hg],
                "wv": wvs[hg],
                "wo": wos[hg],
                "ident": ident,
                "maskb": maskb,
                "bq": bqs[hg],
                "bk": bks[hg],
            }
        )

    res = run_bass_kernel_spmd(nc, in_maps, list(range(NCORES)))
    out = np.empty((B, S, D), np.float32)
    for b in range(B):
        out[b] = res.results[2 * b]["out"] + res.results[2 * b + 1]["out"]
    const = Wo @ bv + bo  # bv/bo contribution (folds exactly through softmax)
    if np.any(const):
        out += const[None, None, :]
    return out
